# revision 36
# baseline (speedup 1.0000x reference)
"""Grouped SwiGLU FFN (8 experts) — expert-parallel Bass kernel for 8 trn2 cores.

Per core (one expert): out = (silu(x@w1) * (x@w3T)) @ w2T.
  x: [T=1024, D=2048], w1: [D, H=4096], w3: [H, D], w2: [D, H].

All matmul operands are float16 — measured fastest PE mode on trn2
(216 ns per 512-col matmul vs 226.5 fp32r, 259 bf16; fp8 would be 2x
but fails the 2e-2 tolerance at ~4-6e-2), psum stays fp32; rel err
~6e-4. Zero on-device transposes — layouts pre-packed on host:
  phase1: g^T[h, t]  = silu(w1^T-tile.T @ x^T) * (w3-tile.T @ x^T)  (per h-tile)
  phase2: out^T[d,t] = sum_h w2-tile.T @ g^T                        (w2 stationary)
Phase1 runs all 32 h-tiles into one full-size g buffer (64KB/partition
fp16); phase2 then streams 16 d-tiles, each as two 32-deep psum
accumulation groups drained via small fp16 stage tiles to DRAM — no
SBUF out accumulator and no vector adds, so only one copy+DMA trails
the last matmul. The measured stream runs gap-free at the 216ns/matmul
PE floor (3072 matmuls ≈ 663.5us); the only other costs are the fixed
~8us queue/DMA spin-up and ~4us drain+teardown. Head tricks: w1/w3
packed into one DRAM tensor per h-tile; h-tile 0's weights and x dt0
ride a single "head bundle" DMA (one transfer + one semaphore — the
DMA engines round-robin across in-flight transfers, so first-needed
data must not share the wire); ~13 dummy matmuls on memset tiles keep
the PE p-state ramping through the spin-up (an idle gap drops the
clock to 1.2GHz for ~3us); the warm start opens all 8 psum banks and
interleaves w1/w3 x th0/th1 per dt-chunk so the PE consumption rate
(~864ns/chunk) stays under the DMA arrival rate (~716ns/chunk).
Engine-level notes: only nc.sync dma_start uses the fast DGE ring
(gpsimd/scalar dma_start land on slow rings); sem-chained DMA pacing
costs ~3us per link and is never worth it. Device DVFS wanders between
~2.37GHz and ~2.0GHz run-to-run (thermal); back-to-back runs throttle.
"""

import sys

sys.path.insert(0, "/opt/trn_rl_repo")

import numpy as np

import concourse.bass as bass
from concourse import bacc
import concourse.mybir as mybir
import concourse.tile as tile
from concourse.bass_utils import run_bass_kernel_spmd

E, T, D, H = 8, 1024, 2048, 4096
P = 128
NT = 512            # matmul moving free dim (fp32 psum max)
DT = D // P         # 16 contraction tiles over D
HT = H // P         # 32 h-tiles
TH = T // NT        # 2 t-halves
DTT = D // P        # 16 out^T row tiles
F32 = mybir.dt.float32
F16 = mybir.dt.float16

_CACHE: dict = {}


def _build_nc():
    nc = bacc.Bacc("TRN2", target_bir_lowering=False, debug=False)
    xp = nc.dram_tensor("xp", [DT, P, T], F16, kind="ExternalInput")
    # dt 8..15 again, packed as two 4-dt quads for single-DMA loads
    xq = nc.dram_tensor("xq", [2, P, 4, T], F16, kind="ExternalInput")
    # head bundle: w1/w3 of h-tile 0 (flattened [2, DT, P] = 4096 els)
    # followed by x dt0 (T els) — one transfer, one semaphore, so the
    # first real matmul's data isn't starved by round-robin DMA sharing
    hd0 = nc.dram_tensor("hd0", [P, 2 * DT * P + T], F16, kind="ExternalInput")
    # w1/w3 packed: wp[ht, p, 0, dt, j] = w1[dt*128+p, ht*128+j]
    #               wp[ht, p, 1, dt, j] = w3[ht*128+j, dt*128+p]
    wp = nc.dram_tensor("wp", [HT, P, 2, DT, P], F16, kind="ExternalInput")
    # w2p[dtt, p, ht, j] = w2[dtt*128+j, ht*128+p]
    w2p = nc.dram_tensor("w2p", [DTT, P, HT, P], F16, kind="ExternalInput")
    outT = nc.dram_tensor("outT", [D, T], F16, kind="ExternalOutput")

    with tile.TileContext(nc) as tc:
        with (
            tc.tile_pool(name="xpool", bufs=1) as xpool,
            tc.tile_pool(name="gpool", bufs=1) as gpool,
            tc.tile_pool(name="wpool", bufs=4) as wpool,
            tc.tile_pool(name="w2pool", bufs=2) as w2pool,
            tc.tile_pool(name="spool", bufs=2) as spool,
            tc.tile_pool(name="pspool", bufs=8, space="PSUM") as pspool,
        ):
            def load_w(ht):
                wsb = wpool.tile([P, 2, DT, P], F16, tag="w", name=f"wsb_{ht}")
                nc.sync.dma_start(wsb, wp[ht])
                return wsb

            # PE pstate warmup: dummy accumulation groups on engine-memset
            # tiles run during the ~9us DMA spin-up so the first real
            # matmuls start with the clock already ramping. Memsets go on
            # gpsimd (its queue comes up ~1us before vector's) and short
            # groups give a fine-grained handoff to real work.
            wrm_w = spool.tile([P, P], F16, tag="wrmw", bufs=1)
            wrm_x = spool.tile([P, NT], F16, tag="wrmx", bufs=1)
            nc.gpsimd.memset(wrm_w, 1.0)
            nc.gpsimd.memset(wrm_x, 1.0)
            # ~13 dummy matmuls cover the window until the head bundle
            # lands (~13us) so the real stream starts at full clock —
            # a PE idle gap drops the clock to the mid p-state (1.2GHz)
            # for ~3us of re-ramp. Later groups are single-matmul so the
            # first real matmul slots in right when its data lands
            # instead of waiting out a 2-matmul group.
            for n in (3, 2, 2) + (1,) * 6:
                psw = pspool.tile([P, NT], F32, tag="ps", bufs=4, name="psw")
                for i in range(n):
                    nc.tensor.matmul(
                        psw, lhsT=wrm_w, rhs=wrm_x,
                        start=(i == 0), stop=(i == n - 1),
                    )

            # head: COARSE transfers — each dma_start costs ~0.6us of DGE
            # issue time, so fewer+bigger beats fine-grained splitting.
            # Order feeds the warm-start: first h-tile weights, then the
            # x chunks it consumes, interleaved with the next weight tiles.
            xsb = xpool.tile([P, DT, T], F16, tag="x")

            # All loads issue on the sync queue (the only fast DGE path —
            # other engines' dma_start lands on slow rings). dt0-7 load
            # per-dt so the warm start's first bursts aren't gated on
            # coarse transfers; dt8-15 as two 1MB quads to keep the issue
            # count (~0.65us apiece, serialized) below the wire time.
            whd = spool.tile([P, 2 * DT * P + T], F16, tag="hd", bufs=1)
            nc.sync.dma_start(whd, hd0[:, :])

            def load_x(dts):
                for dt_i in dts:
                    nc.sync.dma_start(xsb[:, dt_i, :], xp[dt_i])

            load_x(range(1, 8))
            w_stash = {1: load_w(1)}
            nc.sync.dma_start(xsb[:, 8:12, :], xq[0])
            nc.sync.dma_start(xsb[:, 12:16, :], xq[1])
            w_stash[2] = load_w(2)
            w_stash[3] = load_w(3)

            g = gpool.tile([P, HT, T], F16, tag="g")

            WX = 2 * DT * P  # x dt0 offset inside the head bundle

            def xv(dt_i, th):
                if dt_i == 0:
                    return whd[:, WX + th * NT : WX + (th + 1) * NT]
                return xsb[:, dt_i, th * NT : (th + 1) * NT]

            def wv(ht, k, dt_i):
                if ht == 0:
                    return whd[:, (k * DT + dt_i) * P : (k * DT + dt_i + 1) * P]
                return w_stash[ht][:, k, dt_i] if ht in w_stash else None

            def mm_burst(ps, wf, th, dts):
                for dt_i in dts:
                    nc.tensor.matmul(
                        ps,
                        lhsT=wf(dt_i),
                        rhs=xv(dt_i, th),
                        start=(dt_i == 0),
                        stop=(dt_i == DT - 1),
                    )

            def mm_burst4(pss, ht, dts):
                # all four of an h-tile's groups (w1/w3 x th0/th1)
                # interleaved per dt: quarters the per-x-chunk PE
                # consumption rate at the head (864ns/chunk at full
                # clock vs ~716ns DMA arrival) so the warm start never
                # outruns the wire
                for dt_i in dts:
                    for th in range(TH):
                        for k in range(2):
                            nc.tensor.matmul(
                                pss[2 * th + k],
                                lhsT=wv(ht, k, dt_i),
                                rhs=xv(dt_i, th),
                                start=(dt_i == 0),
                                stop=(dt_i == DT - 1),
                            )

            def epilogue(ps1, ps3, ht, th):
                ts = slice(th * NT, (th + 1) * NT)
                sil = spool.tile([P, NT], F32, tag="sil")
                nc.scalar.activation(
                    sil, ps1, mybir.ActivationFunctionType.Silu
                )
                nc.vector.tensor_mul(out=g[:, ht, ts], in0=sil, in1=ps3)

            # warm start: split the first two h-tiles' accumulation into
            # dt halves so 8 psum groups are in the PE queue while the x
            # chunks stream in. ht1's groups borrow the (idle) phase-2
            # "po" psum banks so all 8 groups are open at once — without
            # this the ht1 bursts wait on ht0's completion (bank reuse),
            # which in turn waits on the full x transfer.
            warm = {}
            for ht in range(2):
                ptag = "ps" if ht == 0 else "po"
                pss = [
                    pspool.tile([P, NT], F32, tag=ptag, bufs=4, name=f"w{ht}_{i}")
                    for i in range(4)
                ]
                warm[ht] = pss
                mm_burst4(pss, ht, range(DT // 2))
            for ht in range(2):
                pss = warm[ht]
                for th in range(TH):
                    ps1, ps3 = pss[2 * th], pss[2 * th + 1]
                    mm_burst(ps1, lambda dt, h=ht: wv(h, 0, dt), th, range(DT // 2, DT))
                    mm_burst(ps3, lambda dt, h=ht: wv(h, 1, dt), th, range(DT // 2, DT))
                    epilogue(ps1, ps3, ht, th)

            # phase 1 steady state
            for ht in range(2, HT):
                wsb = w_stash[ht] if ht in w_stash else load_w(ht)
                for th in range(TH):
                    ps1 = pspool.tile([P, NT], F32, tag="ps", bufs=4, name="ps1")
                    ps3 = pspool.tile([P, NT], F32, tag="ps", bufs=4, name="ps3")
                    mm_burst(ps1, lambda dt: wsb[:, 0, dt], th, range(DT))
                    mm_burst(ps3, lambda dt: wsb[:, 1, dt], th, range(DT))
                    epilogue(ps1, ps3, ht, th)

            # phase 2: per d-tile, one 32-deep accumulation group per
            # t-half, drained through a small fp16 stage tile to DRAM
            # (sequential th groups: the th0 drain hides under the th1
            # group, so only one copy+DMA trails the last matmul)
            for dtt in range(DTT):
                w2sb = w2pool.tile([P, HT, P], F16, tag="w2")
                nc.sync.dma_start(w2sb, w2p[dtt])
                for th in range(TH):
                    ts = slice(th * NT, (th + 1) * NT)
                    po = pspool.tile([P, NT], F32, tag="po", bufs=4, name="po")
                    for ht in range(HT):
                        nc.tensor.matmul(
                            po,
                            lhsT=w2sb[:, ht],
                            rhs=g[:, ht, ts],
                            start=(ht == 0),
                            stop=(ht == HT - 1),
                        )
                    st = spool.tile([P, NT], F16, tag="st", bufs=3)
                    nc.vector.tensor_copy(out=st, in_=po)
                    nc.sync.dma_start(
                        outT[dtt * P : (dtt + 1) * P, ts], st
                    )
    nc.compile()
    return nc


def _pack_inputs(x, w1, w2, w3):
    """Per-expert host-side packing into DMA-linear layouts."""
    in_maps = []
    for e in range(E):
        xe = np.asarray(x[e], dtype=np.float32).astype(np.float16)
        w1e = np.asarray(w1[e], dtype=np.float32).astype(np.float16)
        w2e = np.asarray(w2[e], dtype=np.float32).astype(np.float16)
        w3e = np.asarray(w3[e], dtype=np.float32).astype(np.float16)
        # xp[dt, p, t] = x[t, dt*128+p]
        xp = np.ascontiguousarray(xe.reshape(T, DT, P).transpose(1, 2, 0))
        # xq[j, p, i, t] = x[t, (8+j*4+i)*128+p]
        xq = np.ascontiguousarray(
            xe.reshape(T, 4, 4, P)[:, 2:4].transpose(1, 3, 2, 0)
        )
        # w1 part: [ht, p, dt, j] = w1[dt*128+p, ht*128+j]
        w1p = w1e.reshape(DT, P, HT, P).transpose(2, 1, 0, 3)
        # w3 part: [ht, p, dt, j] = w3[ht*128+j, dt*128+p]
        w3p = w3e.reshape(HT, P, DT, P).transpose(0, 3, 2, 1)
        # packed: wp[ht, p, 2, dt, j]
        wpk = np.ascontiguousarray(np.stack([w1p, w3p], axis=2))
        # head bundle: ht0 weights (flattened) + x dt0
        hd0 = np.ascontiguousarray(
            np.concatenate([wpk[0].reshape(P, 2 * DT * P), xp[0]], axis=1)
        )
        # w2p[dtt, p, ht, j] = w2[dtt*128+j, ht*128+p]
        w2p = np.ascontiguousarray(
            w2e.reshape(DTT, P, HT, P).transpose(0, 3, 2, 1)
        )
        in_maps.append({"xp": xp, "xq": xq, "wp": wpk, "w2p": w2p, "hd0": hd0})
    return in_maps


def kernel(x, w1, w2, w3, _trace=False, _trace_kwargs=None):
    if "nc" not in _CACHE:
        _CACHE["nc"] = _build_nc()
    nc = _CACHE["nc"]
    in_maps = _pack_inputs(x, w1, w2, w3)
    kw = {}
    if _trace:
        kw = {"trace": True}
        if _trace_kwargs:
            kw.update(_trace_kwargs)
    res = run_bass_kernel_spmd(nc, in_maps, core_ids=list(range(E)), **kw)
    out = np.empty((E, T, D), dtype=np.float32)
    for e in range(E):
        out[e] = res.results[e]["outT"].astype(np.float32).T
    if _trace:
        _CACHE["last_results"] = res
    return out


# revision 42
# speedup vs baseline: 1.0011x; 1.0011x over previous
"""Grouped SwiGLU FFN (8 experts) — expert-parallel Bass kernel for 8 trn2 cores.

Per core (one expert): out = (silu(x@w1) * (x@w3T)) @ w2T.
  x: [T=1024, D=2048], w1: [D, H=4096], w3: [H, D], w2: [D, H].

All matmul operands are float16 — measured fastest PE mode on trn2
(216 ns per 512-col matmul vs 226.5 fp32r, 259 bf16; fp8 would be 2x
but fails the 2e-2 tolerance at ~4-6e-2), psum stays fp32; rel err
~6e-4. Zero on-device transposes — layouts pre-packed on host:
  phase1: g^T[h, t]  = silu(w1^T-tile.T @ x^T) * (w3-tile.T @ x^T)  (per h-tile)
  phase2: out^T[d,t] = sum_h w2-tile.T @ g^T                        (w2 stationary)
Phase1 runs all 32 h-tiles into one full-size g buffer (64KB/partition
fp16); phase2 then streams 16 d-tiles, each as two 32-deep psum
accumulation groups drained via small fp16 stage tiles to DRAM — no
SBUF out accumulator and no vector adds, so only one copy+DMA trails
the last matmul. The measured stream runs gap-free at the 216ns/matmul
PE floor (3072 matmuls ≈ 663.5us); the only other costs are the fixed
~8us queue/DMA spin-up and ~4us drain+teardown. Head tricks: w1/w3
packed into one DRAM tensor per h-tile; h-tile 0's weights and x dt0
ride a single "head bundle" DMA (one transfer + one semaphore — the
DMA engines round-robin across in-flight transfers, so first-needed
data must not share the wire); ~13 dummy matmuls on memset tiles keep
the PE p-state ramping through the spin-up (an idle gap drops the
clock to 1.2GHz for ~3us); the warm start opens all 8 psum banks and
interleaves w1/w3 x th0/th1 per dt-chunk so the PE consumption rate
(~864ns/chunk) stays under the DMA arrival rate (~716ns/chunk).
Engine-level notes: only nc.sync dma_start uses the fast DGE ring
(gpsimd/scalar dma_start land on slow rings); sem-chained DMA pacing
costs ~3us per link and is never worth it. Device DVFS wanders between
~2.37GHz and ~2.0GHz run-to-run (thermal); back-to-back runs throttle.
"""

import sys

sys.path.insert(0, "/opt/trn_rl_repo")

import numpy as np

import concourse.bass as bass
from concourse import bacc
import concourse.mybir as mybir
import concourse.tile as tile
from concourse.bass_utils import run_bass_kernel_spmd

E, T, D, H = 8, 1024, 2048, 4096
P = 128
NT = 512            # matmul moving free dim (fp32 psum max)
DT = D // P         # 16 contraction tiles over D
HT = H // P         # 32 h-tiles
TH = T // NT        # 2 t-halves
DTT = D // P        # 16 out^T row tiles
F32 = mybir.dt.float32
F16 = mybir.dt.float16

_CACHE: dict = {}


def _build_nc():
    nc = bacc.Bacc("TRN2", target_bir_lowering=False, debug=False)
    xp = nc.dram_tensor("xp", [DT, P, T], F16, kind="ExternalInput")
    # dt 8..15 again, packed as two 4-dt quads for single-DMA loads
    xq = nc.dram_tensor("xq", [2, P, 4, T], F16, kind="ExternalInput")
    # head bundles: the critical 512KB (h-tile 0's dt0-3 weight slices +
    # x dt0) rides one transfer with one semaphore so the first real
    # matmuls aren't starved by round-robin DMA sharing; ht0's dt4-15
    # weights (768KB) follow as a second transfer consumed ~3us later
    hd0 = nc.dram_tensor("hd0", [P, 2 * 4 * P + T], F16, kind="ExternalInput")
    hd1 = nc.dram_tensor("hd1", [P, 2 * 12 * P], F16, kind="ExternalInput")
    # w1/w3 packed: wp[ht, p, 0, dt, j] = w1[dt*128+p, ht*128+j]
    #               wp[ht, p, 1, dt, j] = w3[ht*128+j, dt*128+p]
    wp = nc.dram_tensor("wp", [HT, P, 2, DT, P], F16, kind="ExternalInput")
    # w2p[dtt, p, ht, j] = w2[dtt*128+j, ht*128+p]
    w2p = nc.dram_tensor("w2p", [DTT, P, HT, P], F16, kind="ExternalInput")
    outT = nc.dram_tensor("outT", [D, T], F16, kind="ExternalOutput")

    with tile.TileContext(nc) as tc:
        with (
            tc.tile_pool(name="xpool", bufs=1) as xpool,
            tc.tile_pool(name="gpool", bufs=1) as gpool,
            tc.tile_pool(name="wpool", bufs=4) as wpool,
            tc.tile_pool(name="w2pool", bufs=2) as w2pool,
            tc.tile_pool(name="spool", bufs=2) as spool,
            tc.tile_pool(name="pspool", bufs=8, space="PSUM") as pspool,
        ):
            def load_w(ht):
                wsb = wpool.tile([P, 2, DT, P], F16, tag="w", name=f"wsb_{ht}")
                nc.sync.dma_start(wsb, wp[ht])
                return wsb

            # PE pstate warmup: dummy accumulation groups on engine-memset
            # tiles run during the ~9us DMA spin-up so the first real
            # matmuls start with the clock already ramping. Memsets go on
            # gpsimd (its queue comes up ~1us before vector's) and short
            # groups give a fine-grained handoff to real work.
            wrm_w = spool.tile([P, P], F16, tag="wrmw", bufs=1)
            wrm_x = spool.tile([P, NT], F16, tag="wrmx", bufs=1)
            nc.gpsimd.memset(wrm_w, 1.0)
            nc.gpsimd.memset(wrm_x, 1.0)
            # ~13 dummy matmuls cover the window until the head bundle
            # lands (~13us) so the real stream starts at full clock —
            # a PE idle gap drops the clock to the mid p-state (1.2GHz)
            # for ~3us of re-ramp. Later groups are single-matmul so the
            # first real matmul slots in right when its data lands
            # instead of waiting out a 2-matmul group.
            for n in (3, 2, 2) + (1,) * 3:
                psw = pspool.tile([P, NT], F32, tag="ps", bufs=4, name="psw")
                for i in range(n):
                    nc.tensor.matmul(
                        psw, lhsT=wrm_w, rhs=wrm_x,
                        start=(i == 0), stop=(i == n - 1),
                    )

            # head: COARSE transfers — each dma_start costs ~0.6us of DGE
            # issue time, so fewer+bigger beats fine-grained splitting.
            # Order feeds the warm-start: first h-tile weights, then the
            # x chunks it consumes, interleaved with the next weight tiles.
            xsb = xpool.tile([P, DT, T], F16, tag="x")

            # All loads issue on the sync queue (the only fast DGE path —
            # other engines' dma_start lands on slow rings). dt0-7 load
            # per-dt so the warm start's first bursts aren't gated on
            # coarse transfers; dt8-15 as two 1MB quads to keep the issue
            # count (~0.65us apiece, serialized) below the wire time.
            whd = spool.tile([P, 2 * 4 * P + T], F16, tag="hd", bufs=1)
            nc.sync.dma_start(whd, hd0[:, :])
            whd1 = spool.tile([P, 2 * 12 * P], F16, tag="hd1", bufs=1)
            nc.sync.dma_start(whd1, hd1[:, :])

            def load_x(dts):
                for dt_i in dts:
                    nc.sync.dma_start(xsb[:, dt_i, :], xp[dt_i])

            load_x(range(1, 8))
            w_stash = {1: load_w(1)}
            nc.sync.dma_start(xsb[:, 8:12, :], xq[0])
            nc.sync.dma_start(xsb[:, 12:16, :], xq[1])
            w_stash[2] = load_w(2)
            w_stash[3] = load_w(3)

            g = gpool.tile([P, HT, T], F16, tag="g")

            WX = 2 * 4 * P  # x dt0 offset inside the head bundle

            def xv(dt_i, th):
                if dt_i == 0:
                    return whd[:, WX + th * NT : WX + (th + 1) * NT]
                return xsb[:, dt_i, th * NT : (th + 1) * NT]

            def wv(ht, k, dt_i):
                if ht == 0:
                    if dt_i < 4:
                        o = (k * 4 + dt_i) * P
                        return whd[:, o : o + P]
                    o = (k * 12 + dt_i - 4) * P
                    return whd1[:, o : o + P]
                return w_stash[ht][:, k, dt_i] if ht in w_stash else None

            def mm_burst(ps, wf, th, dts):
                for dt_i in dts:
                    nc.tensor.matmul(
                        ps,
                        lhsT=wf(dt_i),
                        rhs=xv(dt_i, th),
                        start=(dt_i == 0),
                        stop=(dt_i == DT - 1),
                    )

            def mm_burst4(pss, ht, dts):
                # all four of an h-tile's groups (w1/w3 x th0/th1)
                # interleaved per dt: quarters the per-x-chunk PE
                # consumption rate at the head (864ns/chunk at full
                # clock vs ~716ns DMA arrival) so the warm start never
                # outruns the wire
                for dt_i in dts:
                    for th in range(TH):
                        for k in range(2):
                            nc.tensor.matmul(
                                pss[2 * th + k],
                                lhsT=wv(ht, k, dt_i),
                                rhs=xv(dt_i, th),
                                start=(dt_i == 0),
                                stop=(dt_i == DT - 1),
                            )

            def epilogue(ps1, ps3, ht, th):
                ts = slice(th * NT, (th + 1) * NT)
                sil = spool.tile([P, NT], F32, tag="sil")
                nc.scalar.activation(
                    sil, ps1, mybir.ActivationFunctionType.Silu
                )
                nc.vector.tensor_mul(out=g[:, ht, ts], in0=sil, in1=ps3)

            # warm start: split the first two h-tiles' accumulation into
            # dt halves so 8 psum groups are in the PE queue while the x
            # chunks stream in. ht1's groups borrow the (idle) phase-2
            # "po" psum banks so all 8 groups are open at once — without
            # this the ht1 bursts wait on ht0's completion (bank reuse),
            # which in turn waits on the full x transfer.
            warm = {}
            for ht in range(2):
                ptag = "ps" if ht == 0 else "po"
                pss = [
                    pspool.tile([P, NT], F32, tag=ptag, bufs=4, name=f"w{ht}_{i}")
                    for i in range(4)
                ]
                warm[ht] = pss
                mm_burst4(pss, ht, range(DT // 2))
            for ht in range(2):
                pss = warm[ht]
                for th in range(TH):
                    ps1, ps3 = pss[2 * th], pss[2 * th + 1]
                    mm_burst(ps1, lambda dt, h=ht: wv(h, 0, dt), th, range(DT // 2, DT))
                    mm_burst(ps3, lambda dt, h=ht: wv(h, 1, dt), th, range(DT // 2, DT))
                    epilogue(ps1, ps3, ht, th)

            # phase 1 steady state
            for ht in range(2, HT):
                wsb = w_stash[ht] if ht in w_stash else load_w(ht)
                for th in range(TH):
                    ps1 = pspool.tile([P, NT], F32, tag="ps", bufs=4, name="ps1")
                    ps3 = pspool.tile([P, NT], F32, tag="ps", bufs=4, name="ps3")
                    mm_burst(ps1, lambda dt: wsb[:, 0, dt], th, range(DT))
                    mm_burst(ps3, lambda dt: wsb[:, 1, dt], th, range(DT))
                    epilogue(ps1, ps3, ht, th)

            # phase 2: per d-tile, one 32-deep accumulation group per
            # t-half, drained through a small fp16 stage tile to DRAM
            # (sequential th groups: the th0 drain hides under the th1
            # group, so only one copy+DMA trails the last matmul)
            for dtt in range(DTT):
                w2sb = w2pool.tile([P, HT, P], F16, tag="w2")
                nc.sync.dma_start(w2sb, w2p[dtt])
                for th in range(TH):
                    ts = slice(th * NT, (th + 1) * NT)
                    po = pspool.tile([P, NT], F32, tag="po", bufs=4, name="po")
                    for ht in range(HT):
                        nc.tensor.matmul(
                            po,
                            lhsT=w2sb[:, ht],
                            rhs=g[:, ht, ts],
                            start=(ht == 0),
                            stop=(ht == HT - 1),
                        )
                    st = spool.tile([P, NT], F16, tag="st", bufs=3)
                    nc.vector.tensor_copy(out=st, in_=po)
                    nc.sync.dma_start(
                        outT[dtt * P : (dtt + 1) * P, ts], st
                    )
    nc.compile()
    return nc


def _pack_inputs(x, w1, w2, w3):
    """Per-expert host-side packing into DMA-linear layouts."""
    in_maps = []
    for e in range(E):
        xe = np.asarray(x[e], dtype=np.float32).astype(np.float16)
        w1e = np.asarray(w1[e], dtype=np.float32).astype(np.float16)
        w2e = np.asarray(w2[e], dtype=np.float32).astype(np.float16)
        w3e = np.asarray(w3[e], dtype=np.float32).astype(np.float16)
        # xp[dt, p, t] = x[t, dt*128+p]
        xp = np.ascontiguousarray(xe.reshape(T, DT, P).transpose(1, 2, 0))
        # xq[j, p, i, t] = x[t, (8+j*4+i)*128+p]
        xq = np.ascontiguousarray(
            xe.reshape(T, 4, 4, P)[:, 2:4].transpose(1, 3, 2, 0)
        )
        # w1 part: [ht, p, dt, j] = w1[dt*128+p, ht*128+j]
        w1p = w1e.reshape(DT, P, HT, P).transpose(2, 1, 0, 3)
        # w3 part: [ht, p, dt, j] = w3[ht*128+j, dt*128+p]
        w3p = w3e.reshape(HT, P, DT, P).transpose(0, 3, 2, 1)
        # packed: wp[ht, p, 2, dt, j]
        wpk = np.ascontiguousarray(np.stack([w1p, w3p], axis=2))
        # head bundles: ht0 dt0-3 weights + x dt0, then ht0 dt4-15 weights
        hd0 = np.ascontiguousarray(
            np.concatenate(
                [wpk[0][:, :, 0:4, :].reshape(P, 2 * 4 * P), xp[0]], axis=1
            )
        )
        hd1 = np.ascontiguousarray(wpk[0][:, :, 4:16, :].reshape(P, 2 * 12 * P))
        # w2p[dtt, p, ht, j] = w2[dtt*128+j, ht*128+p]
        w2p = np.ascontiguousarray(
            w2e.reshape(DTT, P, HT, P).transpose(0, 3, 2, 1)
        )
        in_maps.append(
            {"xp": xp, "xq": xq, "wp": wpk, "w2p": w2p, "hd0": hd0, "hd1": hd1}
        )
    return in_maps


def kernel(x, w1, w2, w3, _trace=False, _trace_kwargs=None):
    if "nc" not in _CACHE:
        _CACHE["nc"] = _build_nc()
    nc = _CACHE["nc"]
    in_maps = _pack_inputs(x, w1, w2, w3)
    kw = {}
    if _trace:
        kw = {"trace": True}
        if _trace_kwargs:
            kw.update(_trace_kwargs)
    res = run_bass_kernel_spmd(nc, in_maps, core_ids=list(range(E)), **kw)
    out = np.empty((E, T, D), dtype=np.float32)
    for e in range(E):
        out[e] = res.results[e]["outT"].astype(np.float32).T
    if _trace:
        _CACHE["last_results"] = res
    return out
